# revision 1
# baseline (speedup 1.0000x reference)
"""Fused QKV projection (dense transformer attention prologue) on 8 TRN2 NeuronCores.

Reference computation:
    qkv = hidden_states @ concat([Wq, Wk, Wv], axis=1) + concat([bq, bk, bv])
    q, k, v = split(qkv) -> each reshaped to [B, H, S, D] = [4, 16, 4096, 64]

Strategy: data-parallel over tokens (B*S = 16384 tokens -> 2048 per core),
which minimizes per-core HBM traffic (x slice 8 MiB + replicated W 12 MiB +
y slice 24 MiB = 44 MiB/core) vs head-sharded tensor parallelism (~90 MiB).

Each core computes y^T[f, tok] = W^T x^T + b for its token slice:
  - W (fp32 in DRAM) is cast fp32->bf16 inline by the SWDGE DMA load, in
    [128, 768] column chunks so the first matmuls can start early.
  - x loads fp32 on the two HWDGE rings, is cast to bf16 by a DVE copy,
    then transposed with PE identity transposes in bf16 (1 cycle/row vs 2
    for fp32 — halves the transpose cost on the critical engine).
    Transposes for token groups 1..3 are emitted lazily inside phase 0 so
    they hide in the matmul stream.
  - Matmuls run in bf16 with fp32 PSUM accumulation (K=1024 = 8 k-tiles,
    N=512 = one PSUM bank); y^T orientation puts the fused output features
    on partitions, so the bias lands as a per-partition scalar.
  - The bias add is fused into the PSUM eviction (DVE tensor_scalar_add),
    costing nothing extra; y streams out in [128, 512] chunks.
Host side only shards / concatenates / reassembles layouts.

Cost-model exec time ~178.6 us/core (PE busy 170.4 us = 95% occupancy,
zero mid-kernel gaps >300ns; remaining overhead is the x-arrival ramp and
the kernel drain barrier); validated on HW via work-scaling slope (~150-196
us per repeated GEMM phase across runs vs ~165 us modeled).
"""

import numpy as np

import concourse.bass as bass
import concourse.mybir as mybir
from concourse import bacc
from concourse.bass import ds, ts
from concourse.bass_utils import run_bass_kernel_spmd
from concourse.masks import make_identity
from concourse.tile import TileContext

# Problem shapes (hardcoded per contract; kernel.py must be self-contained).
B, S = 4, 4096
HID = 1024
NH, HD = 16, 64
F = 3 * HID              # 3072 fused output features
NCORES = 8
TOK = B * S              # 16384
TOK_PC = TOK // NCORES   # 2048 tokens per core

P = 128
KT = HID // P            # 8 k tiles
XT = TOK_PC // P         # 16 x token tiles
NG = TOK_PC // 512       # 4 token groups of 512 (matmul N)
FT = F // P              # 24 f-tiles total
FH = 768                 # W column chunk (f per DMA)
NH_W = F // FH           # 4 W column chunks
FTH = FH // P            # 3 f-tiles per W chunk

FP32 = mybir.dt.float32
BF16 = mybir.dt.bfloat16


def _build_nc(repeat: int = 1) -> bass.Bass:
    # Bacc (not raw Bass): its compile() runs move_matmul_waits_to_ldweights /
    # generate_event_semaphores, which walrus needs (1 sync-wait per inst).
    # `repeat` replays the main GEMM phase (benchmark-only work scaling).
    nc = bacc.Bacc("TRN2")
    x = nc.declare_dram_parameter("x", [TOK_PC, HID], FP32, isOutput=False)
    w = nc.declare_dram_parameter("w", [HID, F], FP32, isOutput=False)
    bvec = nc.declare_dram_parameter("bvec", [F], FP32, isOutput=False)
    y = nc.declare_dram_parameter("y", [F, TOK_PC], FP32, isOutput=True)

    with TileContext(nc) as tc:
        with (
            tc.tile_pool(name="const", bufs=1) as const_pool,
            tc.tile_pool(name="xin", bufs=6) as x_pool,
            tc.tile_pool(name="xbf", bufs=XT) as xbf_pool,
            tc.tile_pool(name="xtp", bufs=KT * NG) as xt_pool,
            tc.tile_pool(name="wsb", bufs=KT * NH_W) as w_pool,
            tc.tile_pool(name="ysb", bufs=8) as y_pool,
            tc.tile_pool(name="pstr", bufs=2, space="PSUM") as pstr_pool,
            tc.tile_pool(name="psmm", bufs=6, space="PSUM") as psmm_pool,
        ):
            # --- constants -------------------------------------------------
            ident = const_pool.tile([P, P], FP32, name="ident")
            make_identity(nc, ident)
            # bf16 identity for the x transposes: a bf16 transpose streams at
            # 1 cycle/row on the PE vs 2 for fp32 — halves the transpose cost
            identb = const_pool.tile([P, P], BF16, name="identb")
            make_identity(nc, identb)

            # bias laid out [partition, f_tile]: bias_sb[p, f] = bvec[f*128+p].
            # One contiguous [24, 128] DMA, then a PE transpose (K=24) into
            # PSUM and a DVE copy — lands in ~2us instead of 24 tiny DMAs.
            bias_rows = const_pool.tile([FT, P], FP32, name="bias_rows")
            nc.scalar.dma_start(
                out=bias_rows, in_=bvec.rearrange("(f p) -> f p", p=P)
            )
            bias_sb = const_pool.tile([P, FT], FP32, name="bias_sb")
            ps_b = pstr_pool.tile([P, 512], FP32, name="ps_bias", tag="pstr")
            nc.tensor.transpose(ps_b[:, :FT], bias_rows, ident[:FT, :FT])
            nc.vector.tensor_copy(bias_sb, ps_b[:, :FT])

            # --- input DMAs ------------------------------------------------
            # x token tiles [128, 1024] fp32, alternating the two HWDGE rings
            # (SP / ACT) so the early tiles land ~2x sooner than one FIFO.
            # fp32 load (HWDGE, alternating rings), then a DVE cast to bf16.
            # The fp32 staging tile is released right after the cast; the PE
            # transposes read the bf16 copy at half the fp32 streaming cost.
            def _x_dma(t):
                xt = x_pool.tile([P, HID], FP32, name=f"x{t}", tag="x")
                xb = xbf_pool.tile([P, HID], BF16, name=f"xb{t}", tag="xb")
                eng = nc.sync if t % 2 == 0 else nc.scalar
                eng.dma_start(out=xt, in_=x[ts(t, P), :])
                nc.vector.tensor_copy(xb, xt)
                return xb

            # Token group 0 with half-tile granularity, all column-half-0
            # DMAs first (spread over both HWDGE rings): the x-major
            # transpose batches consume k 0..3 (= half 0) of all four tiles
            # first, so the PE starts ~2us earlier.
            H2 = HID // 2
            x_tiles = []
            xg0_f32 = []
            for t in range(4):
                xg0_f32.append(x_pool.tile([P, HID], FP32, name=f"x{t}", tag="x"))
                x_tiles.append(xbf_pool.tile([P, HID], BF16, name=f"xb{t}", tag="xb"))
            for h in range(2):
                cols = ds(h * H2, H2)
                for t in range(4):
                    eng = nc.sync if (t + h) % 2 == 0 else nc.scalar
                    eng.dma_start(out=xg0_f32[t][:, cols], in_=x[ts(t, P), cols])
                for t in range(4):
                    nc.vector.tensor_copy(x_tiles[t][:, cols], xg0_f32[t][:, cols])


            # W tiles per (k, column-chunk): [128, 768] bf16, cast fp32->bf16
            # inline (SWDGE). First chunk (f 0:768, all 8 k) ships first so
            # f=0..5 matmuls can start early.
            w_half = {}

            def _w_dma(k, h):
                wt = w_pool.tile([P, FH], BF16, name=f"w{k}h{h}", tag="w")
                nc.gpsimd.dma_start(out=wt, in_=w[ts(k, P), ds(h * FH, FH)])
                w_half[(k, h)] = wt

            for k in range(KT):
                _w_dma(k, 0)

            x_tiles += [_x_dma(t) for t in range(4, XT)]

            for h in range(1, NH_W):
                for k in range(KT):
                    _w_dma(k, h)

            # --- x transpose ----------------------------------------------
            # xT tile (k, g) holds x^T[k*128:(k+1)*128, g*512:(g+1)*512] bf16.
            xT = {}

            def _transpose_group(g, x_major=False):
                # x_major: iterate source tiles outermost (half the k range
                # at a time so only 4 pstr banks are open) — the PE never
                # stalls waiting for the later x tiles of the group.
                ps_of, bf_of = {}, {}
                for k in range(KT):
                    bf_of[k] = xt_pool.tile(
                        [P, 512], BF16, name=f"xT{g}_{k}", tag="xT"
                    )
                k_batches = (
                    [range(0, 4), range(4, 8)] if x_major else [range(KT)]
                )
                for ks in k_batches:
                    for k in ks:
                        ps_of[k] = pstr_pool.tile(
                            [P, 512], BF16, name=f"ps{g}_{k}", tag="pstr"
                        )
                    if x_major:
                        for i in range(4):
                            for k in ks:
                                nc.tensor.transpose(
                                    ps_of[k][:, ts(i, P)],
                                    x_tiles[4 * g + i][:, ts(k, P)],
                                    identb,
                                )
                    else:
                        for k in ks:
                            for i in range(4):
                                nc.tensor.transpose(
                                    ps_of[k][:, ts(i, P)],
                                    x_tiles[4 * g + i][:, ts(k, P)],
                                    identb,
                                )
                    for k in ks:
                        nc.vector.tensor_copy(bf_of[k], ps_of[k])
                for k in range(KT):
                    xT[(k, g)] = bf_of[k]

            # Group 0 up front (x-major so it starts as soon as x0 lands);
            # groups 1..3 are emitted lazily inside phase 0 so their PSUM
            # evictions interleave with the y evictions on the DVE FIFO.
            _transpose_group(0, x_major=True)
            lazy_pts = {3: 1, 9: 2, 15: 3}

            # --- main GEMM + fused bias + store ----------------------------
            # token-group-outer: phase g sweeps all 24 f-tiles for one group
            # of 512 tokens; xT for group g is only needed at phase g, so the
            # later transposes hide inside phase 0's matmul stream.
            for rep in range(repeat):
                for g in range(NG):
                    for f in range(FT):
                        acc = psmm_pool.tile(
                            [P, 512], FP32, name=f"acc{g}_{f}", tag="acc"
                        )
                        for k in range(KT):
                            nc.tensor.matmul(
                                acc,
                                w_half[(k, f // FTH)][:, ts(f % FTH, P)],
                                xT[(k, g)],
                                start=(k == 0),
                                stop=(k == KT - 1),
                            )
                        # PSUM -> SBUF eviction with fused per-partition bias,
                        # then the [128, 512] chunk streams straight out. The
                        # very last chunk is split in half so its eviction and
                        # store pipeline instead of serializing in the tail.
                        ych = y_pool.tile([P, 512], FP32, name=f"y{g}_{f}", tag="y")
                        last = g == NG - 1 and f == FT - 1 and rep == repeat - 1
                        parts = ((0, 256), (256, 256)) if last else ((0, 512),)
                        for c0, cn in parts:
                            nc.vector.tensor_scalar_add(
                                ych[:, ds(c0, cn)],
                                acc[:, ds(c0, cn)],
                                bias_sb[:, f : f + 1],
                            )
                            nc.scalar.dma_start(
                                out=y[ts(f, P), ds(g * 512 + c0, cn)],
                                in_=ych[:, ds(c0, cn)],
                            )
                        if rep == 0 and g == 0 and f in lazy_pts:
                            _transpose_group(lazy_pts[f])

    nc.finalize()  # runs Bacc.compile(): reg alloc + sync-wait legalization
    return nc


_NC_CACHE = {}

# test-harness hooks: set TRACE=True before calling kernel() to profile the
# run; the full BassKernelResults lands in LAST_RESULTS either way.
TRACE = False
LAST_RESULTS = None

# cached jitted executable: re-running run_bass_kernel_spmd builds a fresh
# executable for the same NEFF each call, and the SECOND execution wedges
# the device (NRT_EXEC_UNIT_UNRECOVERABLE). Building the shard_map'd jit
# once and reusing it is stable across many calls (validated in bench.py).
_RUNNER = None


def _get_nc(repeat: int = 1) -> bass.Bass:
    if repeat not in _NC_CACHE:
        _NC_CACHE[repeat] = _build_nc(repeat)
    return _NC_CACHE[repeat]


def _get_runner():
    global _RUNNER
    if _RUNNER is None:
        import jax
        from jax.sharding import Mesh, PartitionSpec

        try:
            from jax.shard_map import shard_map
        except ImportError:  # older jax
            from jax.experimental.shard_map import shard_map
        from concourse import bass2jax

        nc = _get_nc()
        bass2jax.install_neuronx_cc_hook()
        pname = nc.partition_id_tensor.name if nc.partition_id_tensor else None
        in_names, out_names, out_avals = [], [], []
        for alloc in nc.m.functions[0].allocations:
            if not isinstance(alloc, mybir.MemoryLocationSet):
                continue
            name = alloc.memorylocations[0].name
            if alloc.kind == "ExternalInput":
                if name != pname:
                    in_names.append(name)
            elif alloc.kind == "ExternalOutput":
                out_names.append(name)
                out_avals.append(
                    jax.core.ShapedArray(
                        tuple(alloc.tensor_shape), mybir.dt.np(alloc.dtype)
                    )
                )
        all_in = list(in_names) + list(out_names) + ([pname] if pname else [])

        def _body(*args):
            operands = list(args)
            if pname is not None:
                operands.append(bass2jax.partition_id_tensor())
            return tuple(
                bass2jax._bass_exec_p.bind(
                    *operands,
                    out_avals=tuple(out_avals),
                    in_names=tuple(all_in),
                    out_names=tuple(out_names),
                    lowering_input_output_aliases=(),
                    sim_require_finite=True,
                    sim_require_nnan=True,
                    nc=nc,
                )
            )

        devices = jax.devices()[:NCORES]
        mesh = Mesh(np.asarray(devices), ("core",))
        nspec = len(in_names) + len(out_names)
        fn = jax.jit(
            shard_map(
                _body,
                mesh=mesh,
                in_specs=(PartitionSpec("core"),) * nspec,
                out_specs=(PartitionSpec("core"),) * len(out_names),
                check_rep=False,
            ),
            keep_unused=True,
        )
        _RUNNER = (fn, in_names, out_names, out_avals)
    return _RUNNER


def kernel(hidden_states, Wq, bq, Wk, bk, Wv, bv):
    hidden_states = np.asarray(hidden_states, dtype=np.float32)
    w = np.concatenate(
        [np.asarray(Wq, np.float32), np.asarray(Wk, np.float32), np.asarray(Wv, np.float32)],
        axis=1,
    )
    bvec = np.concatenate(
        [np.asarray(bq, np.float32), np.asarray(bk, np.float32), np.asarray(bv, np.float32)]
    )

    x = np.ascontiguousarray(hidden_states.reshape(TOK, HID))

    if TRACE:
        # dev-only path (profiling hooks); not multi-call-safe
        in_maps = [
            {"x": x[c * TOK_PC : (c + 1) * TOK_PC], "w": w, "bvec": bvec}
            for c in range(NCORES)
        ]
        res = run_bass_kernel_spmd(
            _get_nc(), in_maps, list(range(NCORES)), trace=True
        )
        global LAST_RESULTS
        LAST_RESULTS = res
        outs = res.results
    else:
        fn, in_names, out_names, out_avals = _get_runner()
        per_core = {
            "x": [x[c * TOK_PC : (c + 1) * TOK_PC] for c in range(NCORES)],
            "w": [w] * NCORES,
            "bvec": [bvec] * NCORES,
        }
        concat_in = [np.concatenate(per_core[n], axis=0) for n in in_names]
        concat_zeros = [
            np.zeros((NCORES * a.shape[0], *a.shape[1:]), a.dtype)
            for a in out_avals
        ]
        out = fn(*concat_in, *concat_zeros)
        yi = out_names.index("y")
        y_all = np.asarray(out[yi]).reshape(NCORES, F, TOK_PC)
        outs = [{"y": y_all[c]} for c in range(NCORES)]

    q = np.empty((B, NH, S, HD), np.float32)
    k = np.empty((B, NH, S, HD), np.float32)
    v = np.empty((B, NH, S, HD), np.float32)
    for c in range(NCORES):
        yT = np.asarray(outs[c]["y"])             # [3072, 2048]
        part = yT.reshape(3, NH, HD, TOK_PC)      # [qkv, h, d, tok]
        b_i, s_i = divmod(c, S // TOK_PC)
        s0 = s_i * TOK_PC
        q[b_i, :, s0 : s0 + TOK_PC, :] = part[0].transpose(0, 2, 1)
        k[b_i, :, s0 : s0 + TOK_PC, :] = part[1].transpose(0, 2, 1)
        v[b_i, :, s0 : s0 + TOK_PC, :] = part[2].transpose(0, 2, 1)
    return q, k, v



# revision 3
# speedup vs baseline: 1.3190x; 1.3190x over previous
"""Fused QKV projection (dense transformer attention prologue) on 8 TRN2 NeuronCores.

Reference computation:
    qkv = hidden_states @ concat([Wq, Wk, Wv], axis=1) + concat([bq, bk, bv])
    q, k, v = split(qkv) -> each reshaped to [B, H, S, D] = [4, 16, 4096, 64]

Strategy: data-parallel over tokens (B*S = 16384 tokens -> 2048 per core),
which minimizes per-core HBM traffic vs head-sharded tensor parallelism.

The GEMM runs in fp8 (e4m3) with MatmulPerfMode.DoubleRow: one matmul
instruction contracts TWO k-tiles (stationary [128,2,128], moving
[128,2,512]) at 0.5 cycles/row -- 4x the bf16 MAC rate. Accuracy is
recovered with a 3-term hi/lo split computed on the host:

    x8  = e4m3(x)          xr8 = e4m3(x - x8)        (moving,   scale 1)
    W8  = e4m3(32*W)       Wr8 = e4m3(32*W - W8)     (stationary, scale 32)
    acc = x8@W8 + xr8@W8 + x8@Wr8          (fp32 PSUM, 12 DoubleRow mm/tile)
    y   = (acc + 32*b) * (1/32)            (fused DVE eviction)

The dropped xr@Wr term and the fp8 representation error give rel-l2 err
~1.3e-3 on the graded inputs (measured), far under the 2e-2 gate, while PE
time drops from 170us (bf16, 1.0 cyc/row + on-device transposes) to
96 tiles x 12 mm x 256 cyc = 122.9us. x is pre-transposed on the host so
the device does no transposes at all.

Queue plan: x groups on the SP HWDGE ring, W chunks alternating Act/Pool,
evictions on DVE, y stores alternating SP/Act. Every queue stays well under
the PE's 123us. An early PE transpose (bias layout) warms the p-state ramp.
Host side only quantizes / shards / reassembles layouts.
"""

import numpy as np

import concourse.bass as bass
import concourse.mybir as mybir
from concourse import bacc
from concourse.bass import ds, ts
from concourse.bass_utils import run_bass_kernel_spmd
from concourse.masks import make_identity
from concourse.tile import TileContext

# Problem shapes (hardcoded per contract; kernel.py must be self-contained).
B, S = 4, 4096
HID = 1024
NH, HD = 16, 64
F = 3 * HID              # 3072 fused output features
NCORES = 8
TOK = B * S              # 16384
TOK_PC = TOK // NCORES   # 2048 tokens per core

P = 128
KT = HID // P            # 8 k-tiles per pass
KT2 = 2 * KT             # 16 k-slots (8 main + 8 residual)
NPAIR = KT // 2          # 4 DoubleRow pairs per term
XT = TOK_PC // P         # 16 x token tiles
NG = TOK_PC // 512       # 4 token groups of 512 (matmul N)
FT = F // P              # 24 f-tiles total
FCH = 4                  # W column chunks
FH = F // FCH            # 768 f per W chunk
FTH = FH // P            # 6 f-tiles per W chunk

FP32 = mybir.dt.float32
F8 = mybir.dt.float8e4
DR = mybir.MatmulPerfMode.DoubleRow

WSCALE = 32.0            # W quantized at scale 32 (power of 2: exact in fp32)


def _build_nc() -> bass.Bass:
    # Bacc (not raw Bass): its compile() runs move_matmul_waits_to_ldweights /
    # generate_event_semaphores, which walrus needs (1 sync-wait per inst).
    nc = bacc.Bacc("TRN2")
    # xq[p, g, s, n]: s in 0..7 -> x8 k-tile s, s in 8..15 -> xr8 k-tile s-8;
    # value = q(x)^T[128*k + p, 512*g + n]  (token-major transposed on host)
    xq = nc.declare_dram_parameter("xq", [P, NG, KT2, 512], F8, isOutput=False)
    # wq[p, c, s, j]: same k-slot layout; value = q(32W)[128*k + p, 768*c + j]
    wq = nc.declare_dram_parameter("wq", [P, FCH, KT2, FH], F8, isOutput=False)
    bvec32 = nc.declare_dram_parameter("bvec32", [F], FP32, isOutput=False)
    y = nc.declare_dram_parameter("y", [F, TOK_PC], FP32, isOutput=True)

    with TileContext(nc) as tc:
        with (
            tc.tile_pool(name="const", bufs=1) as const_pool,
            tc.tile_pool(name="wsb", bufs=FCH) as w_pool,
            tc.tile_pool(name="xsb", bufs=NG) as x_pool,
            tc.tile_pool(name="ysb", bufs=8) as y_pool,
            tc.tile_pool(name="pstr", bufs=1, space="PSUM") as pstr_pool,
            tc.tile_pool(name="psmm", bufs=6, space="PSUM") as psmm_pool,
        ):
            # --- constants -------------------------------------------------
            ident = const_pool.tile([P, P], FP32, name="ident")
            make_identity(nc, ident)

            # bias laid out [partition, f_tile]: bias_sb[p, f] = 32*b[f*128+p].
            # One contiguous [24, 128] DMA, then a PE transpose into PSUM and
            # a DVE copy. The transpose also starts the PE p-state ramp clock
            # ~6us before the matmul stream begins.
            bias_rows = const_pool.tile([FT, P], FP32, name="bias_rows")
            nc.gpsimd.dma_start(
                out=bias_rows, in_=bvec32.rearrange("(f p) -> f p", p=P)
            )
            bias_sb = const_pool.tile([P, FT], FP32, name="bias_sb")
            ps_b = pstr_pool.tile([P, 512], FP32, name="ps_bias", tag="pstr")
            nc.tensor.transpose(ps_b[:, :FT], bias_rows, ident[:FT, :FT])
            nc.vector.tensor_copy(bias_sb, ps_b[:, :FT])

            # --- input DMAs ------------------------------------------------
            # x token-group tiles [128, 16, 512] fp8 on the SP ring; W chunks
            # [128, 16, 768] alternating Act / Pool rings so chunk 0 and 1
            # land concurrently. First needed: x g0 + W c0.
            x_sb = []
            for g in range(NG):
                xt = x_pool.tile([P, KT2, 512], F8, name=f"x{g}", tag="x")
                nc.sync.dma_start(out=xt, in_=xq[:, g])
                x_sb.append(xt)

            w_sb = []
            for c in range(FCH):
                wt = w_pool.tile([P, KT2, FH], F8, name=f"w{c}", tag="w")
                eng = nc.scalar if c % 2 == 0 else nc.gpsimd
                eng.dma_start(out=wt, in_=wq[:, c])
                w_sb.append(wt)

            # --- main GEMM + fused bias/scale + store ----------------------
            # Per (g, f): 12 DoubleRow matmuls (4 per term), each contracting
            # two k-slots: term1 x8@W8 (slots m/m), term2 xr8@W8 (r/m),
            # term3 x8@Wr8 (m/r). All accumulate into one fp32 PSUM bank.
            for g in range(NG):
                for f in range(FT):
                    wt = w_sb[f // FTH]
                    fo = (f % FTH) * P
                    acc = psmm_pool.tile(
                        [P, 512], FP32, name=f"acc{g}_{f}", tag="acc"
                    )
                    terms = (
                        (0, 0),   # x8  @ W8
                        (KT, 0),  # xr8 @ W8
                        (0, KT),  # x8  @ Wr8
                    )
                    for ti, (xs, ws) in enumerate(terms):
                        for t in range(NPAIR):
                            nc.tensor.matmul(
                                acc,
                                wt[:, ds(ws + 2 * t, 2), ds(fo, P)],
                                x_sb[g][:, ds(xs + 2 * t, 2), :],
                                start=(ti == 0 and t == 0),
                                stop=(ti == 2 and t == NPAIR - 1),
                                perf_mode=DR,
                            )
                    # PSUM -> SBUF eviction with fused bias + 1/32 scale,
                    # then the [128, 512] chunk streams out, stores
                    # alternating the SP / Act HWDGE rings. The very last
                    # chunk is split in half so its eviction and store
                    # pipeline instead of serializing in the tail.
                    ych = y_pool.tile([P, 512], FP32, name=f"y{g}_{f}", tag="y")
                    last = g == NG - 1 and f == FT - 1
                    parts = ((0, 256), (256, 256)) if last else ((0, 512),)
                    st_eng = nc.sync if (g * FT + f) % 2 == 0 else nc.scalar
                    for c0, cn in parts:
                        nc.vector.tensor_scalar(
                            ych[:, ds(c0, cn)],
                            acc[:, ds(c0, cn)],
                            bias_sb[:, f : f + 1],
                            1.0 / WSCALE,
                            mybir.AluOpType.add,
                            mybir.AluOpType.mult,
                        )
                        st_eng.dma_start(
                            out=y[ts(f, P), ds(g * 512 + c0, cn)],
                            in_=ych[:, ds(c0, cn)],
                        )

    nc.finalize()  # runs Bacc.compile(): reg alloc + sync-wait legalization
    return nc


_NC_CACHE = {}

# test-harness hooks: set TRACE=True before calling kernel() to profile the
# run; the full BassKernelResults lands in LAST_RESULTS either way.
TRACE = False
LAST_RESULTS = None

# cached jitted executable: re-running run_bass_kernel_spmd builds a fresh
# executable for the same NEFF each call, and the SECOND execution wedges
# the device (NRT_EXEC_UNIT_UNRECOVERABLE). Building the shard_map'd jit
# once and reusing it is stable across many calls.
_RUNNER = None


def _get_nc() -> bass.Bass:
    if "nc" not in _NC_CACHE:
        _NC_CACHE["nc"] = _build_nc()
    return _NC_CACHE["nc"]


def _get_runner():
    global _RUNNER
    if _RUNNER is None:
        import jax
        from jax.sharding import Mesh, PartitionSpec

        try:
            from jax.shard_map import shard_map
        except ImportError:  # older jax
            from jax.experimental.shard_map import shard_map
        from concourse import bass2jax

        nc = _get_nc()
        bass2jax.install_neuronx_cc_hook()
        pname = nc.partition_id_tensor.name if nc.partition_id_tensor else None
        in_names, out_names, out_avals = [], [], []
        for alloc in nc.m.functions[0].allocations:
            if not isinstance(alloc, mybir.MemoryLocationSet):
                continue
            name = alloc.memorylocations[0].name
            if alloc.kind == "ExternalInput":
                if name != pname:
                    in_names.append(name)
            elif alloc.kind == "ExternalOutput":
                out_names.append(name)
                out_avals.append(
                    jax.core.ShapedArray(
                        tuple(alloc.tensor_shape), mybir.dt.np(alloc.dtype)
                    )
                )
        all_in = list(in_names) + list(out_names) + ([pname] if pname else [])

        def _body(*args):
            operands = list(args)
            if pname is not None:
                operands.append(bass2jax.partition_id_tensor())
            return tuple(
                bass2jax._bass_exec_p.bind(
                    *operands,
                    out_avals=tuple(out_avals),
                    in_names=tuple(all_in),
                    out_names=tuple(out_names),
                    lowering_input_output_aliases=(),
                    sim_require_finite=True,
                    sim_require_nnan=True,
                    nc=nc,
                )
            )

        devices = jax.devices()[:NCORES]
        mesh = Mesh(np.asarray(devices), ("core",))
        nspec = len(in_names) + len(out_names)
        fn = jax.jit(
            shard_map(
                _body,
                mesh=mesh,
                in_specs=(PartitionSpec("core"),) * nspec,
                out_specs=(PartitionSpec("core"),) * len(out_names),
                check_rep=False,
            ),
            keep_unused=True,
        )
        _RUNNER = (fn, in_names, out_names, out_avals)
    return _RUNNER


def _quantize_inputs(hidden_states, Wq, bq, Wk, bk, Wv, bv):
    """Host-side prep: fp8 hi/lo split + per-core layout shuffling."""
    e4 = mybir.dt.np(F8)  # ml_dtypes.float8_e4m3

    x = np.ascontiguousarray(
        np.asarray(hidden_states, np.float32).reshape(TOK, HID)
    )
    w = np.concatenate(
        [np.asarray(Wq, np.float32), np.asarray(Wk, np.float32),
         np.asarray(Wv, np.float32)],
        axis=1,
    )
    bvec32 = WSCALE * np.concatenate(
        [np.asarray(bq, np.float32), np.asarray(bk, np.float32),
         np.asarray(bv, np.float32)]
    ).astype(np.float32)

    x8 = x.astype(e4)
    xr8 = (x - x8.astype(np.float32)).astype(e4)
    w5 = WSCALE * w
    w8 = w5.astype(e4)
    wr8 = (w5 - w8.astype(np.float32)).astype(e4)

    # xq[core][p, g, s, n] = q^T[128k+p, 512g+n], s = k (x8) or 8+k (xr8)
    def xlayout(a):  # [TOK, HID] fp8 -> [NCORES, P, NG, KT, 512]
        aT = np.ascontiguousarray(a.T)                    # [HID, TOK]
        return (
            aT.reshape(KT, P, NCORES, NG, 512).transpose(2, 1, 3, 0, 4)
        )

    xq = np.concatenate([xlayout(x8), xlayout(xr8)], axis=3)  # [NC,P,NG,KT2,512]
    xq = np.ascontiguousarray(xq)

    def wlayout(a):  # [HID, F] fp8 -> [P, FCH, KT, FH]
        return a.reshape(KT, P, FCH, FH).transpose(1, 2, 0, 3)

    wq = np.ascontiguousarray(
        np.concatenate([wlayout(w8), wlayout(wr8)], axis=2)
    )  # [P, FCH, KT2, FH]
    return xq, wq, bvec32


def kernel(hidden_states, Wq, bq, Wk, bk, Wv, bv):
    xq, wq, bvec32 = _quantize_inputs(hidden_states, Wq, bq, Wk, bk, Wv, bv)

    if TRACE:
        # dev-only path (profiling hooks); not multi-call-safe
        in_maps = [
            {"xq": xq[c], "wq": wq, "bvec32": bvec32} for c in range(NCORES)
        ]
        res = run_bass_kernel_spmd(
            _get_nc(), in_maps, list(range(NCORES)), trace=True
        )
        global LAST_RESULTS
        LAST_RESULTS = res
        outs = res.results
    else:
        fn, in_names, out_names, out_avals = _get_runner()
        per_core = {
            "xq": [xq[c] for c in range(NCORES)],
            "wq": [wq] * NCORES,
            "bvec32": [bvec32] * NCORES,
        }
        concat_in = [np.concatenate(per_core[n], axis=0) for n in in_names]
        concat_zeros = [
            np.zeros((NCORES * a.shape[0], *a.shape[1:]), a.dtype)
            for a in out_avals
        ]
        out = fn(*concat_in, *concat_zeros)
        yi = out_names.index("y")
        y_all = np.asarray(out[yi]).reshape(NCORES, F, TOK_PC)
        outs = [{"y": y_all[c]} for c in range(NCORES)]

    q = np.empty((B, NH, S, HD), np.float32)
    k = np.empty((B, NH, S, HD), np.float32)
    v = np.empty((B, NH, S, HD), np.float32)
    for c in range(NCORES):
        yT = np.asarray(outs[c]["y"])             # [3072, 2048]
        part = yT.reshape(3, NH, HD, TOK_PC)      # [qkv, h, d, tok]
        b_i, s_i = divmod(c, S // TOK_PC)
        s0 = s_i * TOK_PC
        q[b_i, :, s0 : s0 + TOK_PC, :] = part[0].transpose(0, 2, 1)
        k[b_i, :, s0 : s0 + TOK_PC, :] = part[1].transpose(0, 2, 1)
        v[b_i, :, s0 : s0 + TOK_PC, :] = part[2].transpose(0, 2, 1)
    return q, k, v


# revision 8
# speedup vs baseline: 1.3688x; 1.0377x over previous
"""Fused QKV projection (dense transformer attention prologue) on 8 TRN2 NeuronCores.

Reference computation:
    qkv = hidden_states @ concat([Wq, Wk, Wv], axis=1) + concat([bq, bk, bv])
    q, k, v = split(qkv) -> each reshaped to [B, H, S, D] = [4, 16, 4096, 64]

Strategy: data-parallel over tokens (B*S = 16384 tokens -> 2048 per core),
which minimizes per-core HBM traffic vs head-sharded tensor parallelism.

The GEMM runs in fp8 (e4m3) with MatmulPerfMode.DoubleRow: one matmul
instruction contracts TWO k-tiles (stationary [128,2,128], moving
[128,2,512]) at 0.5 cycles/row -- 4x the bf16 MAC rate. Accuracy is
recovered with a 3-term hi/lo split computed on the host:

    x8  = e4m3(x)          xr8 = e4m3(x - x8)        (moving,   scale 1)
    W8  = e4m3(32*W)       Wr8 = e4m3(32*W - W8)     (stationary, scale 32)
    acc = x8@W8 + xr8@W8 + x8@Wr8          (fp32 PSUM, 12 DoubleRow mm/tile)
    y   = (acc + 32*b) * (1/32)            (fused DVE eviction)

The dropped xr@Wr term and the fp8 representation error give rel-l2 err
~1.3e-3 on the graded inputs (measured), far under the 2e-2 gate, while PE
time drops from 170us (bf16, 1.0 cyc/row + on-device transposes) to
96 tiles x 12 mm x 256 cyc = 122.9us. x is pre-transposed on the host so
the device does no transposes at all.

Queue plan: x groups on the SP HWDGE ring, W chunks alternating Act/Pool,
evictions on DVE, y stores alternating SP/Act. Every queue stays well under
the PE's 123us. An early PE transpose (bias layout) warms the p-state ramp.
Host side only quantizes / shards / reassembles layouts.
"""

import numpy as np

import concourse.bass as bass
import concourse.mybir as mybir
from concourse import bacc
from concourse.bass import ds, ts
from concourse.bass_utils import run_bass_kernel_spmd
from concourse.masks import make_identity
from concourse.tile import TileContext

# Problem shapes (hardcoded per contract; kernel.py must be self-contained).
B, S = 4, 4096
HID = 1024
NH, HD = 16, 64
F = 3 * HID              # 3072 fused output features
NCORES = 8
TOK = B * S              # 16384
TOK_PC = TOK // NCORES   # 2048 tokens per core

P = 128
KT = HID // P            # 8 k-tiles per pass
KT2 = 2 * KT             # 16 k-slots (8 main + 8 residual)
NPAIR = KT // 2          # 4 DoubleRow pairs per term
XT = TOK_PC // P         # 16 x token tiles
NG = TOK_PC // 512       # 4 token groups of 512 (matmul N)
FT = F // P              # 24 f-tiles total
FCH = 4                  # W column chunks
FH = F // FCH            # 768 f per W chunk
FTH = FH // P            # 6 f-tiles per W chunk

FP32 = mybir.dt.float32
F8 = mybir.dt.float8e4
DR = mybir.MatmulPerfMode.DoubleRow

WSCALE = 32.0            # W quantized at scale 32 (power of 2: exact in fp32)


def _build_nc() -> bass.Bass:
    # Bacc (not raw Bass): its compile() runs move_matmul_waits_to_ldweights /
    # generate_event_semaphores, which walrus needs (1 sync-wait per inst).
    nc = bacc.Bacc("TRN2")
    # xq[p, g, s, n]: s in 0..7 -> x8 k-tile s, s in 8..15 -> xr8 k-tile s-8;
    # value = q(x)^T[128*k + p, 512*g + n]  (token-major transposed on host)
    xq = nc.declare_dram_parameter("xq", [P, NG, KT2, 512], F8, isOutput=False)
    # wq[p, c, j, s, m]: f-tile-major within each chunk so a single f-tile
    # [128, 16, 128] is contiguous per partition (startup loads in 790ns
    # pieces); value = q(32W)[128*k + p, 768*c + 128*j + m], s-slot layout
    # as for xq.
    wq = nc.declare_dram_parameter(
        "wq", [P, FCH, FTH, KT2, P], F8, isOutput=False
    )
    bvec32 = nc.declare_dram_parameter("bvec32", [F], FP32, isOutput=False)
    y = nc.declare_dram_parameter("y", [F, TOK_PC], FP32, isOutput=True)

    with TileContext(nc) as tc:
        with (
            tc.tile_pool(name="const", bufs=1) as const_pool,
            tc.tile_pool(name="wsb", bufs=FCH) as w_pool,
            tc.tile_pool(name="xsb", bufs=NG) as x_pool,
            tc.tile_pool(name="ysb", bufs=8) as y_pool,
            tc.tile_pool(name="pstr", bufs=1, space="PSUM") as pstr_pool,
            tc.tile_pool(name="psmm", bufs=6, space="PSUM") as psmm_pool,
        ):
            # --- constants -------------------------------------------------
            ident = const_pool.tile([P, P], FP32, name="ident")
            make_identity(nc, ident)

            # bias laid out [partition, f_tile]: bias_sb[p, f] = 32*b[f*128+p].
            # One contiguous [24, 128] DMA (first in the Pool queue), then a
            # PE transpose into PSUM and a DVE copy.
            bias_rows = const_pool.tile([FT, P], FP32, name="bias_rows")
            nc.gpsimd.dma_start(
                out=bias_rows, in_=bvec32.rearrange("(f p) -> f p", p=P)
            )

            # p-state ramp warmup: the PE clock ramps 0.65 -> 1.2 -> 2.4 GHz
            # over ~3us of sustained activity. A chain of dummy identity
            # transposes keeps the PE busy from ~0.3us (ident ready) until
            # the first operands land (~2.6us), so the matmul stream runs at
            # (nearly) full clock from its first instruction.
            ps_warm = pstr_pool.tile([P, 512], FP32, name="ps_warm", tag="pstr")
            for i in range(11):
                nc.tensor.transpose(ps_warm[:, :P], ident, ident)

            bias_sb = const_pool.tile([P, FT], FP32, name="bias_sb")
            nc.tensor.transpose(ps_warm[:, :FT], bias_rows, ident[:FT, :FT])
            nc.vector.tensor_copy(bias_sb, ps_warm[:, :FT])

            # --- input DMAs ------------------------------------------------
            # First-needed pieces go in 790ns chunks so the PE can start at
            # ~2.6us: SP ring feeds x g0 in four 4-slot pieces, the Act ring
            # feeds W chunk 0 one f-tile at a time. The Pool (SWDGE) ring
            # carries the bias + the remaining three W chunks; x g1..g3
            # follow on SP. Stores later share SP/Act.
            x_sb = []
            for g in range(NG):
                xt = x_pool.tile([P, KT2, 512], F8, name=f"x{g}", tag="x")
                x_sb.append(xt)
            for q in range(4):  # x8 h1, x8 h2, xr8 h1, xr8 h2 of group 0
                sl = ds(4 * q, 4)
                nc.sync.dma_start(out=x_sb[0][:, sl, :], in_=xq[:, 0, sl, :])
            for g in range(1, NG):
                nc.sync.dma_start(out=x_sb[g], in_=xq[:, g])

            w_sb = []
            for c in range(FCH):
                wt = w_pool.tile([P, FTH, KT2, P], F8, name=f"w{c}", tag="w")
                w_sb.append(wt)
            for j in range(FTH):  # chunk 0, one f-tile at a time (Act)
                nc.scalar.dma_start(out=w_sb[0][:, j], in_=wq[:, 0, j])
            for c in range(1, FCH):  # chunks 1..3 whole (Pool)
                nc.gpsimd.dma_start(out=w_sb[c], in_=wq[:, c])

            # --- main GEMM + fused bias/scale + store ----------------------
            # Per (g, f): 12 DoubleRow matmuls (4 per term), each contracting
            # two k-slots: term1 x8@W8 (slots m/m), term2 xr8@W8 (r/m),
            # term3 x8@Wr8 (m/r). All accumulate into one fp32 PSUM bank.
            accs = {}

            def _mm(g, f, term, pairs, start=False, stop=False):
                xs, ws = ((0, 0), (KT, 0), (0, KT))[term]
                wt = w_sb[f // FTH]
                for i, t in enumerate(pairs):
                    nc.tensor.matmul(
                        accs[(g, f)],
                        wt[:, f % FTH, ds(ws + 2 * t, 2), :],
                        x_sb[g][:, ds(xs + 2 * t, 2), :],
                        start=start and i == 0,
                        stop=stop and i == len(pairs) - 1,
                        perf_mode=DR,
                    )

            def _evict(g, f, nsplit=1):
                # PSUM -> SBUF eviction with fused bias + 1/32 scale on DVE,
                # then the chunk streams out alternating the SP / Act rings.
                acc = accs.pop((g, f))
                ych = y_pool.tile([P, 512], FP32, name=f"y{g}_{f}", tag="y")
                engs = (nc.sync, nc.scalar)
                base = (g * FT + f) % 2
                cn = 512 // nsplit
                for c in range(nsplit):
                    nc.vector.tensor_scalar(
                        ych[:, ds(c * cn, cn)],
                        acc[:, ds(c * cn, cn)],
                        bias_sb[:, f : f + 1],
                        1.0 / WSCALE,
                        mybir.AluOpType.add,
                        mybir.AluOpType.mult,
                    )
                    engs[(base + c) % 2].dma_start(
                        out=y[ts(f, P), ds(g * 512 + c * cn, cn)],
                        in_=ych[:, ds(c * cn, cn)],
                    )

            ALL = tuple(range(NPAIR))
            for f in range(FT):
                accs[(0, f)] = psmm_pool.tile(
                    [P, 512], FP32, name=f"acc0_{f}", tag="acc"
                )

            # Group-0 prologue ordered by DMA arrival: terms 1/3 of f0..f2
            # consume the x8 pieces and per-f-tile W pieces as they land
            # (f0's first half runs on the first x8 half alone); term 2
            # (xr8) of f0..f2 closes those groups once xr8 arrives.
            _mm(0, 0, 0, (0, 1), start=True)
            _mm(0, 0, 2, (0, 1))
            _mm(0, 0, 0, (2, 3))
            _mm(0, 0, 2, (2, 3))
            for f in (1, 2):
                _mm(0, f, 0, ALL, start=True)
                _mm(0, f, 2, ALL)
            for f in (0, 1, 2):
                _mm(0, f, 1, ALL, stop=True)
                _evict(0, f)
            for f in range(3, FT):
                _mm(0, f, 0, ALL, start=True)
                _mm(0, f, 1, ALL)
                _mm(0, f, 2, ALL, stop=True)
                _evict(0, f)

            for g in range(1, NG):
                for f in range(FT):
                    accs[(g, f)] = psmm_pool.tile(
                        [P, 512], FP32, name=f"acc{g}_{f}", tag="acc"
                    )
                    _mm(g, f, 0, ALL, start=True)
                    _mm(g, f, 1, ALL)
                    _mm(g, f, 2, ALL, stop=True)
                    # the very last tile evicts in quarters so its eviction
                    # and stores pipeline instead of serializing in the tail
                    last = g == NG - 1 and f == FT - 1
                    _evict(g, f, nsplit=4 if last else 1)

    nc.finalize()  # runs Bacc.compile(): reg alloc + sync-wait legalization
    return nc


_NC_CACHE = {}

# test-harness hooks: set TRACE=True before calling kernel() to profile the
# run; the full BassKernelResults lands in LAST_RESULTS either way.
TRACE = False
LAST_RESULTS = None

# cached jitted executable: re-running run_bass_kernel_spmd builds a fresh
# executable for the same NEFF each call, and the SECOND execution wedges
# the device (NRT_EXEC_UNIT_UNRECOVERABLE). Building the shard_map'd jit
# once and reusing it is stable across many calls.
_RUNNER = None


def _get_nc() -> bass.Bass:
    if "nc" not in _NC_CACHE:
        _NC_CACHE["nc"] = _build_nc()
    return _NC_CACHE["nc"]


def _get_runner():
    global _RUNNER
    if _RUNNER is None:
        import jax
        from jax.sharding import Mesh, PartitionSpec

        try:
            from jax.shard_map import shard_map
        except ImportError:  # older jax
            from jax.experimental.shard_map import shard_map
        from concourse import bass2jax

        nc = _get_nc()
        bass2jax.install_neuronx_cc_hook()
        pname = nc.partition_id_tensor.name if nc.partition_id_tensor else None
        in_names, out_names, out_avals = [], [], []
        for alloc in nc.m.functions[0].allocations:
            if not isinstance(alloc, mybir.MemoryLocationSet):
                continue
            name = alloc.memorylocations[0].name
            if alloc.kind == "ExternalInput":
                if name != pname:
                    in_names.append(name)
            elif alloc.kind == "ExternalOutput":
                out_names.append(name)
                out_avals.append(
                    jax.core.ShapedArray(
                        tuple(alloc.tensor_shape), mybir.dt.np(alloc.dtype)
                    )
                )
        all_in = list(in_names) + list(out_names) + ([pname] if pname else [])

        def _body(*args):
            operands = list(args)
            if pname is not None:
                operands.append(bass2jax.partition_id_tensor())
            return tuple(
                bass2jax._bass_exec_p.bind(
                    *operands,
                    out_avals=tuple(out_avals),
                    in_names=tuple(all_in),
                    out_names=tuple(out_names),
                    lowering_input_output_aliases=(),
                    sim_require_finite=True,
                    sim_require_nnan=True,
                    nc=nc,
                )
            )

        devices = jax.devices()[:NCORES]
        mesh = Mesh(np.asarray(devices), ("core",))
        nspec = len(in_names) + len(out_names)
        fn = jax.jit(
            shard_map(
                _body,
                mesh=mesh,
                in_specs=(PartitionSpec("core"),) * nspec,
                out_specs=(PartitionSpec("core"),) * len(out_names),
                check_rep=False,
            ),
            keep_unused=True,
        )
        _RUNNER = (fn, in_names, out_names, out_avals)
    return _RUNNER


def _quantize_inputs(hidden_states, Wq, bq, Wk, bk, Wv, bv):
    """Host-side prep: fp8 hi/lo split + per-core layout shuffling."""
    e4 = mybir.dt.np(F8)  # ml_dtypes.float8_e4m3

    x = np.ascontiguousarray(
        np.asarray(hidden_states, np.float32).reshape(TOK, HID)
    )
    w = np.concatenate(
        [np.asarray(Wq, np.float32), np.asarray(Wk, np.float32),
         np.asarray(Wv, np.float32)],
        axis=1,
    )
    bvec32 = WSCALE * np.concatenate(
        [np.asarray(bq, np.float32), np.asarray(bk, np.float32),
         np.asarray(bv, np.float32)]
    ).astype(np.float32)

    x8 = x.astype(e4)
    xr8 = (x - x8.astype(np.float32)).astype(e4)
    w5 = WSCALE * w
    w8 = w5.astype(e4)
    wr8 = (w5 - w8.astype(np.float32)).astype(e4)

    # xq[core][p, g, s, n] = q^T[128k+p, 512g+n], s = k (x8) or 8+k (xr8)
    def xlayout(a):  # [TOK, HID] fp8 -> [NCORES, P, NG, KT, 512]
        aT = np.ascontiguousarray(a.T)                    # [HID, TOK]
        return (
            aT.reshape(KT, P, NCORES, NG, 512).transpose(2, 1, 3, 0, 4)
        )

    xq = np.concatenate([xlayout(x8), xlayout(xr8)], axis=3)  # [NC,P,NG,KT2,512]
    xq = np.ascontiguousarray(xq)

    def wlayout(a):  # [HID, F] fp8 -> [P, FCH, FTH, KT, 128]
        return a.reshape(KT, P, FCH, FTH, P).transpose(1, 2, 3, 0, 4)

    wq = np.ascontiguousarray(
        np.concatenate([wlayout(w8), wlayout(wr8)], axis=3)
    )  # [P, FCH, FTH, KT2, 128]
    return xq, wq, bvec32


def kernel(hidden_states, Wq, bq, Wk, bk, Wv, bv):
    xq, wq, bvec32 = _quantize_inputs(hidden_states, Wq, bq, Wk, bk, Wv, bv)

    if TRACE:
        # dev-only path (profiling hooks); not multi-call-safe
        in_maps = [
            {"xq": xq[c], "wq": wq, "bvec32": bvec32} for c in range(NCORES)
        ]
        res = run_bass_kernel_spmd(
            _get_nc(), in_maps, list(range(NCORES)), trace=True
        )
        global LAST_RESULTS
        LAST_RESULTS = res
        outs = res.results
    else:
        fn, in_names, out_names, out_avals = _get_runner()
        per_core = {
            "xq": [xq[c] for c in range(NCORES)],
            "wq": [wq] * NCORES,
            "bvec32": [bvec32] * NCORES,
        }
        concat_in = [np.concatenate(per_core[n], axis=0) for n in in_names]
        concat_zeros = [
            np.zeros((NCORES * a.shape[0], *a.shape[1:]), a.dtype)
            for a in out_avals
        ]
        out = fn(*concat_in, *concat_zeros)
        yi = out_names.index("y")
        y_all = np.asarray(out[yi]).reshape(NCORES, F, TOK_PC)
        outs = [{"y": y_all[c]} for c in range(NCORES)]

    q = np.empty((B, NH, S, HD), np.float32)
    k = np.empty((B, NH, S, HD), np.float32)
    v = np.empty((B, NH, S, HD), np.float32)
    for c in range(NCORES):
        yT = np.asarray(outs[c]["y"])             # [3072, 2048]
        part = yT.reshape(3, NH, HD, TOK_PC)      # [qkv, h, d, tok]
        b_i, s_i = divmod(c, S // TOK_PC)
        s0 = s_i * TOK_PC
        q[b_i, :, s0 : s0 + TOK_PC, :] = part[0].transpose(0, 2, 1)
        k[b_i, :, s0 : s0 + TOK_PC, :] = part[1].transpose(0, 2, 1)
        v[b_i, :, s0 : s0 + TOK_PC, :] = part[2].transpose(0, 2, 1)
    return q, k, v


# revision 11
# speedup vs baseline: 1.4902x; 1.0886x over previous
"""Fused QKV projection (dense transformer attention prologue) on 8 TRN2 NeuronCores.

Reference computation:
    qkv = hidden_states @ concat([Wq, Wk, Wv], axis=1) + concat([bq, bk, bv])
    q, k, v = split(qkv) -> each reshaped to [B, H, S, D] = [4, 16, 4096, 64]

Strategy: data-parallel over tokens (B*S = 16384 tokens -> 2048 per core),
which minimizes per-core HBM traffic vs head-sharded tensor parallelism.

The GEMM runs in fp8 (e4m3) with MatmulPerfMode.DoubleRow: one matmul
instruction contracts TWO k-tiles (stationary [128,2,128], moving
[128,2,512]) at 0.5 cycles/row -- 4x the bf16 MAC rate. Accuracy is
recovered with a 3-term hi/lo split computed on the host:

    x8  = e4m3(x)          xr8 = e4m3(x - x8)        (moving,   scale 1)
    W8  = e4m3(32*W)       Wr8 = e4m3(32*W - W8)     (stationary, scale 32)
    acc = x8@W8 + xr8@W8 + x8@Wr8          (fp32 PSUM, 12 DoubleRow mm/tile)
    y   = (acc + 32*b) * (1/32)            (fused DVE eviction)

The dropped xr@Wr term and the fp8 representation error give rel-l2 err
~1.3e-3 on the graded inputs (measured), far under the 2e-2 gate, while PE
time drops from 170us (bf16, 1.0 cyc/row + on-device transposes) to
96 tiles x 12 mm x 256 cyc = 122.9us. x is pre-transposed on the host so
the device does no transposes at all.

Queue plan: x groups on the SP HWDGE ring, W chunks alternating Act/Pool,
evictions on DVE, y stores alternating SP/Act. Every queue stays well under
the PE's 123us. An early PE transpose (bias layout) warms the p-state ramp.
Host side only quantizes / shards / reassembles layouts.
"""

import numpy as np

import concourse.bass as bass
import concourse.mybir as mybir
from concourse import bacc
from concourse.bass import ds, ts
from concourse.bass_utils import run_bass_kernel_spmd
from concourse.masks import make_identity
from concourse.tile import TileContext

# Problem shapes (hardcoded per contract; kernel.py must be self-contained).
B, S = 4, 4096
HID = 1024
NH, HD = 16, 64
F = 3 * HID              # 3072 fused output features
NCORES = 8
TOK = B * S              # 16384
TOK_PC = TOK // NCORES   # 2048 tokens per core

P = 128
KT = HID // P            # 8 k-tiles per pass
KT2 = 2 * KT             # 16 k-slots (8 main + 8 residual)
NPAIR = KT // 2          # 4 DoubleRow pairs per term
XT = TOK_PC // P         # 16 x token tiles
NG = TOK_PC // 512       # 4 token groups of 512 (matmul N)
FT = F // P              # 24 f-tiles total
FCH = 4                  # W column chunks
FH = F // FCH            # 768 f per W chunk
FTH = FH // P            # 6 f-tiles per W chunk

FP32 = mybir.dt.float32
F8 = mybir.dt.float8e4
DR = mybir.MatmulPerfMode.DoubleRow

WSCALE = 32.0            # W quantized at scale 32 (power of 2: exact in fp32)


def _build_nc() -> bass.Bass:
    # Bacc (not raw Bass): its compile() runs move_matmul_waits_to_ldweights /
    # generate_event_semaphores, which walrus needs (1 sync-wait per inst).
    nc = bacc.Bacc("TRN2")
    # xq[p, g, s, n]: s in 0..7 -> x8 k-tile s, s in 8..15 -> xr8 k-tile s-8;
    # value = q(x)^T[128*k + p, 512*g + n]  (token-major transposed on host)
    xq = nc.declare_dram_parameter("xq", [P, NG, KT2, 512], F8, isOutput=False)
    # wq[p, c, j, s, m]: f-tile-major within each chunk so a single f-tile
    # [128, 16, 128] is contiguous per partition (startup loads in 790ns
    # pieces); value = q(32W)[128*k + p, 768*c + 128*j + m], s-slot layout
    # as for xq.
    wq = nc.declare_dram_parameter(
        "wq", [P, FCH, FTH, KT2, P], F8, isOutput=False
    )
    bvec32 = nc.declare_dram_parameter("bvec32", [F], FP32, isOutput=False)
    y = nc.declare_dram_parameter("y", [F, TOK_PC], FP32, isOutput=True)

    with TileContext(nc) as tc:
        with (
            tc.tile_pool(name="const", bufs=1) as const_pool,
            tc.tile_pool(name="wsb", bufs=FCH) as w_pool,
            tc.tile_pool(name="xsb", bufs=NG) as x_pool,
            tc.tile_pool(name="ysb", bufs=8) as y_pool,
            tc.tile_pool(name="pstr", bufs=1, space="PSUM") as pstr_pool,
            tc.tile_pool(name="psmm", bufs=6, space="PSUM") as psmm_pool,
        ):
            # --- constants -------------------------------------------------
            # make_identity's Pool ops go FIRST on the Pool engine so the PE
            # warmup chain can start ~0.4us; the bias DMA follows.
            ident = const_pool.tile([P, P], FP32, name="ident")
            make_identity(nc, ident)

            # bias laid out [partition, f_tile]: bias_sb[p, f] = 32*b[f*128+p].
            # One contiguous [24, 128] DMA, then a PE transpose into PSUM and
            # a DVE copy.
            bias_rows = const_pool.tile([FT, P], FP32, name="bias_rows")
            nc.gpsimd.dma_start(
                out=bias_rows, in_=bvec32.rearrange("(f p) -> f p", p=P)
            )

            # p-state ramp warmup: the PE clock ramps 0.65 -> 1.2 -> 2.4 GHz
            # over ~3us of sustained activity. A chain of dummy identity
            # transposes keeps the PE busy from ~0.4us (ident ready) until
            # the first operands land (~2.5us), so the matmul stream runs at
            # (nearly) full clock from its first instruction.
            ps_warm = pstr_pool.tile([P, 512], FP32, name="ps_warm", tag="pstr")
            for i in range(10):
                nc.tensor.transpose(ps_warm[:, :P], ident, ident)

            bias_sb = const_pool.tile([P, FT], FP32, name="bias_sb")
            nc.tensor.transpose(ps_warm[:, :FT], bias_rows, ident[:FT, :FT])
            nc.vector.tensor_copy(bias_sb, ps_warm[:, :FT])

            # --- input DMAs ------------------------------------------------
            # First-needed pieces go in 500ns chunks so the PE can start at
            # ~2.5us: SP feeds x8 of group 0 in 2-slot pieces, the Act ring
            # feeds W f-tile 0 as W8-half then Wr8-half, then per-f-tile.
            # The Pool (SWDGE) ring carries ident + bias + W chunks 1..3;
            # x g1..g3 follow on SP. Stores later share SP/Act.
            # xr8 slots 14..15 (k-tiles 6-7 of the x residual) are never read
            # by the 11-inst tiles, so they are not even loaded.
            x_sb = []
            for g in range(NG):
                xt = x_pool.tile([P, KT2, 512], F8, name=f"x{g}", tag="x")
                x_sb.append(xt)
            for sl in (ds(0, 2), ds(2, 2), ds(4, 2), ds(6, 2),
                       ds(8, 3), ds(11, 3)):
                nc.sync.dma_start(out=x_sb[0][:, sl, :], in_=xq[:, 0, sl, :])
            for g in range(1, NG):
                nc.sync.dma_start(
                    out=x_sb[g][:, : KT + 6, :], in_=xq[:, g, : KT + 6, :]
                )

            w_sb = []
            for c in range(FCH):
                wt = w_pool.tile([P, FTH, KT2, P], F8, name=f"w{c}", tag="w")
                w_sb.append(wt)
            nc.scalar.dma_start(out=w_sb[0][:, 0, :KT], in_=wq[:, 0, 0, :KT])
            nc.scalar.dma_start(out=w_sb[0][:, 0, KT:], in_=wq[:, 0, 0, KT:])
            for j in range(1, FTH):  # chunk 0, one f-tile at a time (Act)
                nc.scalar.dma_start(out=w_sb[0][:, j], in_=wq[:, 0, j])
            for c in range(1, FCH):  # chunks 1..3 whole (Pool)
                nc.gpsimd.dma_start(out=w_sb[c], in_=wq[:, c])

            # --- main GEMM + fused bias/scale + store ----------------------
            # Per (g, f): 11 DoubleRow matmuls, each contracting two k-slots:
            #   term0 x8@W8   pairs 0..3 (slots m/m)  -- the main product
            #   term1 xr8@W8  pairs 0..2 (slots r/m)  -- x-residual, k0..k5
            #   term2 x8@Wr8  pairs 0..3 (slots m/r)  -- W-residual, full
            # (the x-residual correction on k6/k7 is dropped: it moves the
            # rel-l2 error from 1.3e-3 to a measured 1.33e-2, still 1.5x
            # under the 2e-2 gate, and saves 1/12 of all PE time.)
            # All 11 accumulate into one fp32 PSUM bank.
            accs = {}
            TERM_PAIRS = (range(NPAIR), range(NPAIR - 1), range(NPAIR))

            def _mm(key, term, pairs, start=False, stop=False, cols=None):
                xs, ws = ((0, 0), (KT, 0), (0, KT))[term]
                g = key[0]
                wt = w_sb[key[1] // FTH]
                csl = ds(0, 512) if cols is None else cols
                for i, t in enumerate(pairs):
                    nc.tensor.matmul(
                        accs[key],
                        wt[:, key[1] % FTH, ds(ws + 2 * t, 2), :],
                        x_sb[g][:, ds(xs + 2 * t, 2), csl],
                        start=start and i == 0,
                        stop=stop and i == len(pairs) - 1,
                        perf_mode=DR,
                    )

            def _evict(key, f, cols, st_par):
                # PSUM -> SBUF eviction with fused bias + 1/32 scale on DVE,
                # then the chunk streams out on the SP / Act rings.
                acc = accs.pop(key)
                g = key[0]
                ych = y_pool.tile([P, cols[1]], FP32, name=f"y{key}", tag="y")
                nc.vector.tensor_scalar(
                    ych,
                    acc[:, : cols[1]] if len(key) > 2 else acc[:, ds(*cols)],
                    bias_sb[:, f : f + 1],
                    1.0 / WSCALE,
                    mybir.AluOpType.add,
                    mybir.AluOpType.mult,
                )
                [nc.sync, nc.scalar][st_par % 2].dma_start(
                    out=y[ts(f, P), ds(g * 512 + cols[0], cols[1])],
                    in_=ych,
                )

            def _tile(g, f, nsplit=1):
                # nsplit > 1: independent column-slice accumulation groups so
                # the final evictions/stores pipeline in the drain.
                cn = 512 // nsplit
                for c in range(nsplit):
                    key = (g, f, c) if nsplit > 1 else (g, f)
                    accs[key] = psmm_pool.tile(
                        [P, cn], FP32, name=f"acc{key}", tag="acc"
                    )
                    cols = ds(c * cn, cn) if nsplit > 1 else None
                    for ti in range(3):
                        _mm(key, ti, TERM_PAIRS[ti],
                            start=(ti == 0), stop=(ti == 2), cols=cols)
                    _evict(key, f, (c * cn, cn), g * FT + f + c)

            # Group-0 prologue ordered by DMA arrival: x8 lands in 2-slot
            # pieces, W f-tile 0 in two halves, f-tiles 1.. behind them; the
            # xr8 term of f0..f2 closes those groups once xr8 lands.
            for f in range(3):
                accs[(0, f)] = psmm_pool.tile(
                    [P, 512], FP32, name=f"acc0_{f}", tag="acc"
                )
            _mm((0, 0), 0, (0,), start=True)
            _mm((0, 0), 2, (0,))
            _mm((0, 0), 0, (1,))
            _mm((0, 0), 2, (1,))
            _mm((0, 1), 0, (0, 1), start=True)
            _mm((0, 0), 0, (2,))
            _mm((0, 0), 2, (2,))
            _mm((0, 1), 2, (0, 1))
            _mm((0, 0), 0, (3,))
            _mm((0, 0), 2, (3,))
            _mm((0, 1), 0, (2, 3))
            _mm((0, 1), 2, (2, 3))
            _mm((0, 2), 0, TERM_PAIRS[0], start=True)
            _mm((0, 2), 2, TERM_PAIRS[2])
            for f in (0, 1, 2):
                _mm((0, f), 1, TERM_PAIRS[1], stop=True)
                _evict((0, f), f, (0, 512), f)
            for f in range(3, FT):
                _tile(0, f)

            for g in range(1, NG):
                for f in range(FT):
                    last = g == NG - 1 and f == FT - 1
                    _tile(g, f, nsplit=4 if last else 1)

    nc.finalize()  # runs Bacc.compile(): reg alloc + sync-wait legalization
    return nc


_NC_CACHE = {}

# test-harness hooks: set TRACE=True before calling kernel() to profile the
# run; the full BassKernelResults lands in LAST_RESULTS either way.
TRACE = False
LAST_RESULTS = None

# cached jitted executable: re-running run_bass_kernel_spmd builds a fresh
# executable for the same NEFF each call, and the SECOND execution wedges
# the device (NRT_EXEC_UNIT_UNRECOVERABLE). Building the shard_map'd jit
# once and reusing it is stable across many calls.
_RUNNER = None


def _get_nc() -> bass.Bass:
    if "nc" not in _NC_CACHE:
        _NC_CACHE["nc"] = _build_nc()
    return _NC_CACHE["nc"]


def _get_runner():
    global _RUNNER
    if _RUNNER is None:
        import jax
        from jax.sharding import Mesh, PartitionSpec

        try:
            from jax.shard_map import shard_map
        except ImportError:  # older jax
            from jax.experimental.shard_map import shard_map
        from concourse import bass2jax

        nc = _get_nc()
        bass2jax.install_neuronx_cc_hook()
        pname = nc.partition_id_tensor.name if nc.partition_id_tensor else None
        in_names, out_names, out_avals = [], [], []
        for alloc in nc.m.functions[0].allocations:
            if not isinstance(alloc, mybir.MemoryLocationSet):
                continue
            name = alloc.memorylocations[0].name
            if alloc.kind == "ExternalInput":
                if name != pname:
                    in_names.append(name)
            elif alloc.kind == "ExternalOutput":
                out_names.append(name)
                out_avals.append(
                    jax.core.ShapedArray(
                        tuple(alloc.tensor_shape), mybir.dt.np(alloc.dtype)
                    )
                )
        all_in = list(in_names) + list(out_names) + ([pname] if pname else [])

        def _body(*args):
            operands = list(args)
            if pname is not None:
                operands.append(bass2jax.partition_id_tensor())
            return tuple(
                bass2jax._bass_exec_p.bind(
                    *operands,
                    out_avals=tuple(out_avals),
                    in_names=tuple(all_in),
                    out_names=tuple(out_names),
                    lowering_input_output_aliases=(),
                    sim_require_finite=True,
                    sim_require_nnan=True,
                    nc=nc,
                )
            )

        devices = jax.devices()[:NCORES]
        mesh = Mesh(np.asarray(devices), ("core",))
        nspec = len(in_names) + len(out_names)
        fn = jax.jit(
            shard_map(
                _body,
                mesh=mesh,
                in_specs=(PartitionSpec("core"),) * nspec,
                out_specs=(PartitionSpec("core"),) * len(out_names),
                check_rep=False,
            ),
            keep_unused=True,
        )
        _RUNNER = (fn, in_names, out_names, out_avals)
    return _RUNNER


def _quantize_inputs(hidden_states, Wq, bq, Wk, bk, Wv, bv):
    """Host-side prep: fp8 hi/lo split + per-core layout shuffling."""
    e4 = mybir.dt.np(F8)  # ml_dtypes.float8_e4m3

    x = np.ascontiguousarray(
        np.asarray(hidden_states, np.float32).reshape(TOK, HID)
    )
    w = np.concatenate(
        [np.asarray(Wq, np.float32), np.asarray(Wk, np.float32),
         np.asarray(Wv, np.float32)],
        axis=1,
    )
    bvec32 = WSCALE * np.concatenate(
        [np.asarray(bq, np.float32), np.asarray(bk, np.float32),
         np.asarray(bv, np.float32)]
    ).astype(np.float32)

    x8 = x.astype(e4)
    xr8 = (x - x8.astype(np.float32)).astype(e4)
    w5 = WSCALE * w
    w8 = w5.astype(e4)
    wr8 = (w5 - w8.astype(np.float32)).astype(e4)

    # xq[core][p, g, s, n] = q^T[128k+p, 512g+n], s = k (x8) or 8+k (xr8)
    def xlayout(a):  # [TOK, HID] fp8 -> [NCORES, P, NG, KT, 512]
        aT = np.ascontiguousarray(a.T)                    # [HID, TOK]
        return (
            aT.reshape(KT, P, NCORES, NG, 512).transpose(2, 1, 3, 0, 4)
        )

    xq = np.concatenate([xlayout(x8), xlayout(xr8)], axis=3)  # [NC,P,NG,KT2,512]
    xq = np.ascontiguousarray(xq)

    def wlayout(a):  # [HID, F] fp8 -> [P, FCH, FTH, KT, 128]
        return a.reshape(KT, P, FCH, FTH, P).transpose(1, 2, 3, 0, 4)

    wq = np.ascontiguousarray(
        np.concatenate([wlayout(w8), wlayout(wr8)], axis=3)
    )  # [P, FCH, FTH, KT2, 128]
    return xq, wq, bvec32


def kernel(hidden_states, Wq, bq, Wk, bk, Wv, bv):
    xq, wq, bvec32 = _quantize_inputs(hidden_states, Wq, bq, Wk, bk, Wv, bv)

    if TRACE:
        # dev-only path (profiling hooks); not multi-call-safe
        in_maps = [
            {"xq": xq[c], "wq": wq, "bvec32": bvec32} for c in range(NCORES)
        ]
        res = run_bass_kernel_spmd(
            _get_nc(), in_maps, list(range(NCORES)), trace=True
        )
        global LAST_RESULTS
        LAST_RESULTS = res
        outs = res.results
    else:
        fn, in_names, out_names, out_avals = _get_runner()
        per_core = {
            "xq": [xq[c] for c in range(NCORES)],
            "wq": [wq] * NCORES,
            "bvec32": [bvec32] * NCORES,
        }
        concat_in = [np.concatenate(per_core[n], axis=0) for n in in_names]
        concat_zeros = [
            np.zeros((NCORES * a.shape[0], *a.shape[1:]), a.dtype)
            for a in out_avals
        ]
        out = fn(*concat_in, *concat_zeros)
        yi = out_names.index("y")
        y_all = np.asarray(out[yi]).reshape(NCORES, F, TOK_PC)
        outs = [{"y": y_all[c]} for c in range(NCORES)]

    q = np.empty((B, NH, S, HD), np.float32)
    k = np.empty((B, NH, S, HD), np.float32)
    v = np.empty((B, NH, S, HD), np.float32)
    for c in range(NCORES):
        yT = np.asarray(outs[c]["y"])             # [3072, 2048]
        part = yT.reshape(3, NH, HD, TOK_PC)      # [qkv, h, d, tok]
        b_i, s_i = divmod(c, S // TOK_PC)
        s0 = s_i * TOK_PC
        q[b_i, :, s0 : s0 + TOK_PC, :] = part[0].transpose(0, 2, 1)
        k[b_i, :, s0 : s0 + TOK_PC, :] = part[1].transpose(0, 2, 1)
        v[b_i, :, s0 : s0 + TOK_PC, :] = part[2].transpose(0, 2, 1)
    return q, k, v


# revision 18
# speedup vs baseline: 1.4999x; 1.0065x over previous
"""Fused QKV projection (dense transformer attention prologue) on 8 TRN2 NeuronCores.

Reference computation:
    qkv = hidden_states @ concat([Wq, Wk, Wv], axis=1) + concat([bq, bk, bv])
    q, k, v = split(qkv) -> each reshaped to [B, H, S, D] = [4, 16, 4096, 64]

Strategy: data-parallel over tokens (B*S = 16384 tokens -> 2048 per core),
which minimizes per-core HBM traffic vs head-sharded tensor parallelism.

The GEMM runs in fp8 (e4m3) with MatmulPerfMode.DoubleRow: one matmul
instruction contracts TWO k-tiles (stationary [128,2,128], moving
[128,2,512]) at 0.5 cycles/row -- 4x the bf16 MAC rate. Accuracy is
recovered with a 3-term hi/lo split computed on the host:

    x8  = e4m3(x)          xr8 = e4m3(x - x8)        (moving,   scale 1)
    W8  = e4m3(32*W)       Wr8 = e4m3(32*W - W8)     (stationary, scale 32)
    acc = x8@W8 + xr8@W8 + x8@Wr8          (fp32 PSUM, 12 DoubleRow mm/tile)
    y   = (acc + 32*b) * (1/32)            (fused DVE eviction)

The dropped xr@Wr term and the fp8 representation error give rel-l2 err
~1.3e-3 on the graded inputs (measured), far under the 2e-2 gate, while PE
time drops from 170us (bf16, 1.0 cyc/row + on-device transposes) to
96 tiles x 12 mm x 256 cyc = 122.9us. x is pre-transposed on the host so
the device does no transposes at all.

Queue plan: x groups on the SP HWDGE ring, W chunks alternating Act/Pool,
evictions on DVE, y stores alternating SP/Act. Every queue stays well under
the PE's 123us. An early PE transpose (bias layout) warms the p-state ramp.
Host side only quantizes / shards / reassembles layouts.
"""

import numpy as np

import concourse.bass as bass
import concourse.mybir as mybir
from concourse import bacc
from concourse.bass import ds, ts
from concourse.bass_utils import run_bass_kernel_spmd
from concourse.masks import make_identity
from concourse.tile import TileContext

# Problem shapes (hardcoded per contract; kernel.py must be self-contained).
B, S = 4, 4096
HID = 1024
NH, HD = 16, 64
F = 3 * HID              # 3072 fused output features
NCORES = 8
TOK = B * S              # 16384
TOK_PC = TOK // NCORES   # 2048 tokens per core

P = 128
KT = HID // P            # 8 k-tiles per pass
KT2 = 2 * KT             # 16 k-slots (8 main + 8 residual)
NPAIR = KT // 2          # 4 DoubleRow pairs per term
XT = TOK_PC // P         # 16 x token tiles
NG = TOK_PC // 512       # 4 token groups of 512 (matmul N)
FT = F // P              # 24 f-tiles total
FCH = 4                  # W column chunks
FH = F // FCH            # 768 f per W chunk
FTH = FH // P            # 6 f-tiles per W chunk

FP32 = mybir.dt.float32
F8 = mybir.dt.float8e4
DR = mybir.MatmulPerfMode.DoubleRow

WSCALE = 32.0            # W quantized at scale 32 (power of 2: exact in fp32)


def _build_nc() -> bass.Bass:
    # Bacc (not raw Bass): its compile() runs move_matmul_waits_to_ldweights /
    # generate_event_semaphores, which walrus needs (1 sync-wait per inst).
    nc = bacc.Bacc("TRN2")
    # xq[p, g, s, n]: s in 0..7 -> x8 k-tile s, s in 8..15 -> xr8 k-tile s-8;
    # value = q(x)^T[128*k + p, 512*g + n]  (token-major transposed on host)
    xq = nc.declare_dram_parameter("xq", [P, NG, KT2, 512], F8, isOutput=False)
    # wq[p, c, j, s, m]: f-tile-major within each chunk so a single f-tile
    # [128, 16, 128] is contiguous per partition (startup loads in 790ns
    # pieces); value = q(32W)[128*k + p, 768*c + 128*j + m], s-slot layout
    # as for xq.
    wq = nc.declare_dram_parameter(
        "wq", [P, FCH, FTH, KT2, P], F8, isOutput=False
    )
    bvec32 = nc.declare_dram_parameter("bvec32", [F], FP32, isOutput=False)
    y = nc.declare_dram_parameter("y", [F, TOK_PC], FP32, isOutput=True)

    with TileContext(nc) as tc:
        with (
            tc.tile_pool(name="const", bufs=1) as const_pool,
            tc.tile_pool(name="wsb", bufs=FCH) as w_pool,
            tc.tile_pool(name="xsb", bufs=NG) as x_pool,
            tc.tile_pool(name="ysb", bufs=8) as y_pool,
            tc.tile_pool(name="pstr", bufs=1, space="PSUM") as pstr_pool,
            tc.tile_pool(name="psmm", bufs=6, space="PSUM") as psmm_pool,
        ):
            # --- constants -------------------------------------------------
            # make_identity's Pool ops go FIRST on the Pool engine so the PE
            # warmup chain can start ~0.4us; the bias DMA follows.
            ident = const_pool.tile([P, P], FP32, name="ident")
            make_identity(nc, ident)

            # bias laid out [partition, f_tile]: bias_sb[p, f] = 32*b[f*128+p].
            # One contiguous [24, 128] DMA, then a PE transpose into PSUM and
            # a DVE copy.
            bias_rows = const_pool.tile([FT, P], FP32, name="bias_rows")
            nc.gpsimd.dma_start(
                out=bias_rows, in_=bvec32.rearrange("(f p) -> f p", p=P)
            )

            # p-state ramp warmup: the PE clock ramps 0.65 -> 1.2 -> 2.4 GHz
            # over ~3us of sustained activity. A chain of dummy identity
            # transposes keeps the PE busy from ~0.4us (ident ready) until
            # the first operands land (~2.5us), so the matmul stream runs at
            # (nearly) full clock from its first instruction.
            ps_warm = pstr_pool.tile([P, 512], FP32, name="ps_warm", tag="pstr")
            for i in range(7):
                nc.tensor.transpose(ps_warm[:, :P], ident, ident)

            bias_sb = const_pool.tile([P, FT], FP32, name="bias_sb")
            nc.tensor.transpose(ps_warm[:, :FT], bias_rows, ident[:FT, :FT])
            nc.vector.tensor_copy(bias_sb, ps_warm[:, :FT])
            # unscaled bias for the one Act-engine eviction (activation
            # computes func(in*scale + bias), so its bias is b, not 32b)
            bias1_sb = const_pool.tile([P, FT], FP32, name="bias1_sb")
            nc.vector.tensor_scalar_mul(bias1_sb, bias_sb, 1.0 / WSCALE)

            # --- input DMAs ------------------------------------------------
            # First-needed pieces go in 500ns chunks so the PE can start at
            # ~2.5us: SP feeds x8 of group 0 in 2-slot pieces, the Act ring
            # feeds W f-tile 0 as W8-half then Wr8-half, then per-f-tile.
            # The Pool (SWDGE) ring carries ident + bias + W chunks 1..3;
            # x g1..g3 follow on SP. Stores later share SP/Act.
            # xr8 slots 14..15 (k-tiles 6-7 of the x residual) are never read
            # by the 11-inst tiles, so they are not even loaded.
            x_sb = []
            for g in range(NG):
                xt = x_pool.tile([P, KT2, 512], F8, name=f"x{g}", tag="x")
                x_sb.append(xt)
            for sl in (ds(0, 2), ds(2, 2), ds(4, 2), ds(6, 2),
                       ds(8, 3), ds(11, 3)):
                nc.sync.dma_start(out=x_sb[0][:, sl, :], in_=xq[:, 0, sl, :])
            for g in range(1, NG):
                nc.sync.dma_start(
                    out=x_sb[g][:, : KT + 6, :], in_=xq[:, g, : KT + 6, :]
                )

            w_sb = []
            for c in range(FCH):
                wt = w_pool.tile([P, FTH, KT2, P], F8, name=f"w{c}", tag="w")
                w_sb.append(wt)
            nc.scalar.dma_start(out=w_sb[0][:, 0, :KT], in_=wq[:, 0, 0, :KT])
            nc.scalar.dma_start(out=w_sb[0][:, 0, KT:], in_=wq[:, 0, 0, KT:])
            for j in range(1, FTH):  # chunk 0, one f-tile at a time (Act)
                nc.scalar.dma_start(out=w_sb[0][:, j], in_=wq[:, 0, j])
            for c in range(1, FCH):  # chunks 1..3 whole (Pool)
                nc.gpsimd.dma_start(out=w_sb[c], in_=wq[:, c])

            # preload the Act engine's Identity activation table during its
            # idle window (between the startup W pieces and the first store)
            # so the one Act-engine eviction in the drain pays no table load
            act_warm = const_pool.tile([P, 1], FP32, name="act_warm")
            nc.scalar.activation(
                act_warm, ident[:, :1], mybir.ActivationFunctionType.Identity
            )

            # --- main GEMM + fused bias/scale + store ----------------------
            # Per (g, f): 11 DoubleRow matmuls, each contracting two k-slots:
            #   term0 x8@W8   pairs 0..3 (slots m/m)  -- the main product
            #   term1 xr8@W8  pairs 0..2 (slots r/m)  -- x-residual, k0..k5
            #   term2 x8@Wr8  pairs 0..3 (slots m/r)  -- W-residual, full
            # (the x-residual correction on k6/k7 is dropped: it moves the
            # rel-l2 error from 1.3e-3 to a measured 1.33e-2, still 1.5x
            # under the 2e-2 gate, and saves 1/12 of all PE time.)
            # All 11 accumulate into one fp32 PSUM bank.
            accs = {}
            TERM_PAIRS = (range(NPAIR), range(NPAIR - 1), range(NPAIR))

            def _mm(key, term, pairs, start=False, stop=False, cols=None):
                xs, ws = ((0, 0), (KT, 0), (0, KT))[term]
                g = key[0]
                wt = w_sb[key[1] // FTH]
                csl = ds(0, 512) if cols is None else cols
                for i, t in enumerate(pairs):
                    nc.tensor.matmul(
                        accs[key],
                        wt[:, key[1] % FTH, ds(ws + 2 * t, 2), :],
                        x_sb[g][:, ds(xs + 2 * t, 2), csl],
                        start=start and i == 0,
                        stop=stop and i == len(pairs) - 1,
                        perf_mode=DR,
                    )

            def _evict(key, f, cols, st_par, ev_eng=None, st_eng=None):
                # PSUM -> SBUF eviction with fused bias + 1/32 scale on DVE,
                # then the chunk streams out on the SP / Act rings.
                acc = accs.pop(key)
                g = key[0]
                ych = y_pool.tile([P, cols[1]], FP32, name=f"y{key}", tag="y")
                src = acc[:, : cols[1]] if len(key) > 2 else acc[:, ds(*cols)]
                if ev_eng is nc.scalar:
                    nc.scalar.activation(
                        ych,
                        src,
                        mybir.ActivationFunctionType.Identity,
                        bias=bias1_sb[:, f : f + 1],
                        scale=1.0 / WSCALE,
                    )
                else:
                    (ev_eng or nc.vector).tensor_scalar(
                        ych,
                        src,
                        bias_sb[:, f : f + 1],
                        1.0 / WSCALE,
                        mybir.AluOpType.add,
                        mybir.AluOpType.mult,
                    )
                (st_eng or [nc.sync, nc.scalar][st_par % 2]).dma_start(
                    out=y[ts(f, P), ds(g * 512 + cols[0], cols[1])],
                    in_=ych,
                )

            def _tile(g, f, nsplit=1):
                # nsplit > 1 (final tile): independent column-slice
                # accumulation groups so the drain pipelines; the last
                # quarter evicts on the otherwise-idle Pool engine and
                # stores on the by-then-idle SP ring.
                cn = 512 // nsplit
                for c in range(nsplit):
                    key = (g, f, c) if nsplit > 1 else (g, f)
                    accs[key] = psmm_pool.tile(
                        [P, cn], FP32, name=f"acc{key}", tag="acc"
                    )
                    cols = ds(c * cn, cn) if nsplit > 1 else None
                    for ti in range(3):
                        _mm(key, ti, TERM_PAIRS[ti],
                            start=(ti == 0), stop=(ti == 2), cols=cols)
                    ev_eng = st_eng = None
                    if nsplit > 1:
                        # drain: quarters evict on DVE back-to-back; their
                        # stores mostly on SP (Act is finishing f22's), the
                        # final one on the by-then-free Act ring
                        st_eng = (nc.sync, nc.sync, nc.sync, nc.scalar)[c % 4]
                    elif (g, f) == (NG - 1, FT - 2):
                        # f22 evicts+stores via the Act engine so DVE and SP
                        # are clear when the final quarters need them
                        ev_eng = st_eng = nc.scalar
                    elif (g, f) == (NG - 1, FT - 3):
                        st_eng = nc.sync
                    _evict(key, f, (c * cn, cn), g * FT + f + c,
                           ev_eng, st_eng)

            # Group-0 prologue ordered by DMA arrival: x8 lands in 2-slot
            # pieces, W f-tile 0 in two halves, f-tiles 1.. behind them; the
            # xr8 term of f0..f2 closes those groups once xr8 lands.
            for f in range(3):
                accs[(0, f)] = psmm_pool.tile(
                    [P, 512], FP32, name=f"acc0_{f}", tag="acc"
                )
            _mm((0, 0), 0, (0,), start=True)
            _mm((0, 0), 2, (0,))
            _mm((0, 0), 0, (1,))
            _mm((0, 0), 2, (1,))
            _mm((0, 1), 0, (0, 1), start=True)
            _mm((0, 0), 0, (2,))
            _mm((0, 0), 2, (2,))
            _mm((0, 1), 2, (0, 1))
            _mm((0, 0), 0, (3,))
            _mm((0, 0), 2, (3,))
            _mm((0, 1), 0, (2, 3))
            _mm((0, 1), 2, (2, 3))
            _mm((0, 2), 0, TERM_PAIRS[0], start=True)
            _mm((0, 2), 2, TERM_PAIRS[2])
            for f in (0, 1, 2):
                _mm((0, f), 1, TERM_PAIRS[1], stop=True)
                _evict((0, f), f, (0, 512), f)
            for f in range(3, FT):
                _tile(0, f)

            for g in range(1, NG):
                for f in range(FT):
                    last = g == NG - 1 and f == FT - 1
                    _tile(g, f, nsplit=4 if last else 1)

    nc.finalize()  # runs Bacc.compile(): reg alloc + sync-wait legalization
    return nc


_NC_CACHE = {}

# test-harness hooks: set TRACE=True before calling kernel() to profile the
# run; the full BassKernelResults lands in LAST_RESULTS either way.
TRACE = False
LAST_RESULTS = None

# cached jitted executable: re-running run_bass_kernel_spmd builds a fresh
# executable for the same NEFF each call, and the SECOND execution wedges
# the device (NRT_EXEC_UNIT_UNRECOVERABLE). Building the shard_map'd jit
# once and reusing it is stable across many calls.
_RUNNER = None


def _get_nc() -> bass.Bass:
    if "nc" not in _NC_CACHE:
        _NC_CACHE["nc"] = _build_nc()
    return _NC_CACHE["nc"]


def _get_runner():
    global _RUNNER
    if _RUNNER is None:
        import jax
        from jax.sharding import Mesh, PartitionSpec

        try:
            from jax.shard_map import shard_map
        except ImportError:  # older jax
            from jax.experimental.shard_map import shard_map
        from concourse import bass2jax

        nc = _get_nc()
        bass2jax.install_neuronx_cc_hook()
        pname = nc.partition_id_tensor.name if nc.partition_id_tensor else None
        in_names, out_names, out_avals = [], [], []
        for alloc in nc.m.functions[0].allocations:
            if not isinstance(alloc, mybir.MemoryLocationSet):
                continue
            name = alloc.memorylocations[0].name
            if alloc.kind == "ExternalInput":
                if name != pname:
                    in_names.append(name)
            elif alloc.kind == "ExternalOutput":
                out_names.append(name)
                out_avals.append(
                    jax.core.ShapedArray(
                        tuple(alloc.tensor_shape), mybir.dt.np(alloc.dtype)
                    )
                )
        all_in = list(in_names) + list(out_names) + ([pname] if pname else [])

        def _body(*args):
            operands = list(args)
            if pname is not None:
                operands.append(bass2jax.partition_id_tensor())
            return tuple(
                bass2jax._bass_exec_p.bind(
                    *operands,
                    out_avals=tuple(out_avals),
                    in_names=tuple(all_in),
                    out_names=tuple(out_names),
                    lowering_input_output_aliases=(),
                    sim_require_finite=True,
                    sim_require_nnan=True,
                    nc=nc,
                )
            )

        devices = jax.devices()[:NCORES]
        mesh = Mesh(np.asarray(devices), ("core",))
        nspec = len(in_names) + len(out_names)
        fn = jax.jit(
            shard_map(
                _body,
                mesh=mesh,
                in_specs=(PartitionSpec("core"),) * nspec,
                out_specs=(PartitionSpec("core"),) * len(out_names),
                check_rep=False,
            ),
            keep_unused=True,
        )
        _RUNNER = (fn, in_names, out_names, out_avals)
    return _RUNNER


def _quantize_inputs(hidden_states, Wq, bq, Wk, bk, Wv, bv):
    """Host-side prep: fp8 hi/lo split + per-core layout shuffling."""
    e4 = mybir.dt.np(F8)  # ml_dtypes.float8_e4m3

    x = np.ascontiguousarray(
        np.asarray(hidden_states, np.float32).reshape(TOK, HID)
    )
    w = np.concatenate(
        [np.asarray(Wq, np.float32), np.asarray(Wk, np.float32),
         np.asarray(Wv, np.float32)],
        axis=1,
    )
    bvec32 = WSCALE * np.concatenate(
        [np.asarray(bq, np.float32), np.asarray(bk, np.float32),
         np.asarray(bv, np.float32)]
    ).astype(np.float32)

    x8 = x.astype(e4)
    xr8 = (x - x8.astype(np.float32)).astype(e4)
    w5 = WSCALE * w
    w8 = w5.astype(e4)
    wr8 = (w5 - w8.astype(np.float32)).astype(e4)

    # xq[core][p, g, s, n] = q^T[128k+p, 512g+n], s = k (x8) or 8+k (xr8)
    def xlayout(a):  # [TOK, HID] fp8 -> [NCORES, P, NG, KT, 512]
        aT = np.ascontiguousarray(a.T)                    # [HID, TOK]
        return (
            aT.reshape(KT, P, NCORES, NG, 512).transpose(2, 1, 3, 0, 4)
        )

    xq = np.concatenate([xlayout(x8), xlayout(xr8)], axis=3)  # [NC,P,NG,KT2,512]
    xq = np.ascontiguousarray(xq)

    def wlayout(a):  # [HID, F] fp8 -> [P, FCH, FTH, KT, 128]
        return a.reshape(KT, P, FCH, FTH, P).transpose(1, 2, 3, 0, 4)

    wq = np.ascontiguousarray(
        np.concatenate([wlayout(w8), wlayout(wr8)], axis=3)
    )  # [P, FCH, FTH, KT2, 128]
    return xq, wq, bvec32


def kernel(hidden_states, Wq, bq, Wk, bk, Wv, bv):
    xq, wq, bvec32 = _quantize_inputs(hidden_states, Wq, bq, Wk, bk, Wv, bv)

    if TRACE:
        # dev-only path (profiling hooks); not multi-call-safe
        in_maps = [
            {"xq": xq[c], "wq": wq, "bvec32": bvec32} for c in range(NCORES)
        ]
        res = run_bass_kernel_spmd(
            _get_nc(), in_maps, list(range(NCORES)), trace=True
        )
        global LAST_RESULTS
        LAST_RESULTS = res
        outs = res.results
    else:
        fn, in_names, out_names, out_avals = _get_runner()
        per_core = {
            "xq": [xq[c] for c in range(NCORES)],
            "wq": [wq] * NCORES,
            "bvec32": [bvec32] * NCORES,
        }
        concat_in = [np.concatenate(per_core[n], axis=0) for n in in_names]
        concat_zeros = [
            np.zeros((NCORES * a.shape[0], *a.shape[1:]), a.dtype)
            for a in out_avals
        ]
        out = fn(*concat_in, *concat_zeros)
        yi = out_names.index("y")
        y_all = np.asarray(out[yi]).reshape(NCORES, F, TOK_PC)
        outs = [{"y": y_all[c]} for c in range(NCORES)]

    q = np.empty((B, NH, S, HD), np.float32)
    k = np.empty((B, NH, S, HD), np.float32)
    v = np.empty((B, NH, S, HD), np.float32)
    for c in range(NCORES):
        yT = np.asarray(outs[c]["y"])             # [3072, 2048]
        part = yT.reshape(3, NH, HD, TOK_PC)      # [qkv, h, d, tok]
        b_i, s_i = divmod(c, S // TOK_PC)
        s0 = s_i * TOK_PC
        q[b_i, :, s0 : s0 + TOK_PC, :] = part[0].transpose(0, 2, 1)
        k[b_i, :, s0 : s0 + TOK_PC, :] = part[1].transpose(0, 2, 1)
        v[b_i, :, s0 : s0 + TOK_PC, :] = part[2].transpose(0, 2, 1)
    return q, k, v
